# revision 18
# baseline (speedup 1.0000x reference)
"""CSNN LIF-scan kernel for Trainium2, 8 NeuronCores.

reference computes:
    cur = x @ W.T + b                      # [128, 10000]
    scan t=0..49:  reset = (mem > 1); mem = 0.95*mem + cur - reset
                   spk = (mem > 1)
    returns spk_rec, mem_rec               # each [50, 128, 10000] f32

Key identities exploited here:
  * spk_rec[t] == (mem_rec[t] > 1) exactly, so only ONE tensor needs to
    leave the device; the host derives spikes from it losslessly.
  * In threshold-shifted state v = mem - 1 the whole step is
        v' = (v*beta + cur') - (v > 0),   cur' = cur + (beta - 1)
    which fits a single fused custom-DVE op (one 1x pass/step) instead of
    two scalar_tensor_tensor passes + a compare.
  * (beta-1) is folded into the matmul bias row on the host, so cur' comes
    straight out of PSUM.
  * v ships as fp16 (cast inline by the SWDGE DMA engines): v is centered
    on the spike threshold, so (v_fp16 > 0) still reproduces the spike
    train bit-exactly away from a ~2^-25 dead band.

Sharding: model-parallel over the neuron axis (10000 = 8 x 1250), x
replicated, W/b sliced per core; batch=128 rides the SBUF partitions.
"""

import sys

for _p in ("/opt/trn_rl_repo", "/root/.axon_site/_ro/trn_rl_repo"):
    if _p not in sys.path:
        sys.path.append(_p)

import numpy as np

import concourse.bass as bass
import concourse.tile as tile
import concourse.dve_ops as dve_ops
from concourse import mybir
from concourse.dve_spec import C0, C1, Spec, Src0, Src1

F32 = mybir.dt.float32
F16 = mybir.dt.float16

N_CORES = 8
B = 128          # batch (SBUF partitions)
AXON = 1000      # contraction dim
K_PAD = 1024     # padded contraction (8 x 128); row 1000 carries the bias
N_TOTAL = 10000
NL = N_TOTAL // N_CORES  # 1250 neurons per core
T = 50
BETA = 0.95
THRESH = 1.0

# matmul free-dim chunks (PSUM bank holds 512 f32)
MM_CHUNKS = [(0, 512), (512, 1024), (1024, 1250)]


def _lif_ref(in0, in1, s0, s1, imm2):
    in0 = np.asarray(in0, np.float32)
    in1 = np.asarray(in1, np.float32)
    return (
        (in0 * np.float32(s0) + in1) - (in0 > np.float32(s1)).astype(np.float32)
    ).astype(np.float32)


def _register_lif_op() -> "dve_ops.DveOp":
    """out = (in0*s0 + in1) - (in0 > s1): one fused LIF step per DVE pass."""
    name = "LIF_STEP_ANT"
    for op in dve_ops.OPS:
        if op.name == name:
            return op
    op = dve_ops.DveOp(
        name,
        Spec(body=(Src0 * C0 + Src1) - (Src0 > C1), reference=_lif_ref),
        subdim=False,
        uops_sha={"v3": "4d971942aba05d49", "v4": "da6677450a1cb1b9"},
    )
    dve_ops.OPS.append(op)
    dve_ops.CUSTOM_DVE_SPECS[name] = op.spec
    dve_ops._SUB_OPCODE_FOR_NAME[name] = (
        dve_ops._CUSTOM_DVE_ROW_BASE + len(dve_ops.OPS) - 1
    )
    assert dve_ops._SUB_OPCODE_FOR_NAME[name] < 0x20
    return op


LIF_OP = _register_lif_op()


def _split_excess_waits(bir: dict) -> int:
    """walrus in this env lowers at most ONE sync-wait per instruction, but
    Tile emits several. Move extras onto injected EventSemaphore carriers
    placed just before the instruction on the same engine."""
    n_split = [0]

    def fix_block(block):
        for inner in block.get("blocks", []):
            fix_block(inner)
        insts = block.get("instructions")
        if not insts:
            return
        new_insts = []
        for inst in insts:
            si = inst.get("sync_info")
            waits = (si or {}).get("on_wait", [])
            if len(waits) > 1:
                for w in waits[:-1]:
                    n_split[0] += 1
                    new_insts.append(
                        {
                            "debug": inst.get("debug", 0),
                            "engine": inst["engine"],
                            "ins": [],
                            "name": f"I-wsplit-{n_split[0]}",
                            "opcode": "EventSemaphore",
                            "outs": [],
                            "sync_info": {"on_update": [], "on_wait": [w]},
                        }
                    )
                si["on_wait"] = [waits[-1]]
            new_insts.append(inst)
        block["instructions"] = new_insts

    for fn in bir.get("functions", []):
        fix_block(fn)
    return n_split[0]


def _patch_serialization(nc: bass.Bass) -> bass.Bass:
    import json as _json
    import types as _types

    orig = nc.to_json_bytes

    def to_json_bytes(self):
        bir = _json.loads(orig())
        _split_excess_waits(bir)
        return _json.dumps(bir).encode()

    nc.to_json_bytes = _types.MethodType(to_json_bytes, nc)
    return nc


def _build_program() -> bass.Bass:
    from contextlib import ExitStack

    nc = bass.Bass()
    # fp16 split-matmul inputs: cur = xh@Wh + xh@Wl + xl@Wh (the dropped
    # xl@Wl term is ~2^-22 relative). PE runs fp16 at 1 cycle/row vs 4 for
    # f32, cutting the pre-scan phase by ~40%.
    xh = nc.dram_tensor("xh", [K_PAD, B], F16, kind="ExternalInput")
    xl = nc.dram_tensor("xl", [K_PAD, B], F16, kind="ExternalInput")
    wh = nc.dram_tensor("wh", [K_PAD, NL], F16, kind="ExternalInput")
    wl = nc.dram_tensor("wl", [K_PAD, NL], F16, kind="ExternalInput")
    v_rec = nc.dram_tensor("v_rec", [T, B, NL], F16, kind="ExternalOutput")

    KT = K_PAD // 128  # 8 contraction tiles

    with tile.TileContext(nc) as tc, ExitStack() as ctx:
        xpool = ctx.enter_context(tc.tile_pool(name="xp", bufs=KT))
        wpool = ctx.enter_context(tc.tile_pool(name="wp", bufs=KT))
        curp = ctx.enter_context(tc.tile_pool(name="curp", bufs=1))
        psum = ctx.enter_context(tc.tile_pool(name="psum", bufs=1, space="PSUM"))
        vpool = ctx.enter_context(tc.tile_pool(name="vp", bufs=12))
        hpool = ctx.enter_context(tc.tile_pool(name="hp", bufs=6))

        # x tensors first (feed LDWEIGHTS), then W per k-tile, hi before lo,
        # all on one HWDGE ring so arrival order matches consumption order.
        xhtile = xpool.tile([128, KT, B], F16, tag="xh")
        nc.sync.dma_start(out=xhtile, in_=xh.rearrange("(k p) m -> p k m", p=128))
        xltile = xpool.tile([128, KT, B], F16, tag="xl")
        nc.sync.dma_start(out=xltile, in_=xl.rearrange("(k p) m -> p k m", p=128))

        wh_v = wh.rearrange("(k p) n -> p k n", p=128)
        wl_v = wl.rearrange("(k p) n -> p k n", p=128)
        wh_tiles, wl_tiles = [], []
        for k in range(KT):
            wgh = wpool.tile([128, NL], F16, tag="wh")
            nc.sync.dma_start(out=wgh, in_=wh_v[:, k])
            wh_tiles.append(wgh)
            wgl = wpool.tile([128, NL], F16, tag="wl")
            nc.sync.dma_start(out=wgl, in_=wl_v[:, k])
            wl_tiles.append(wgl)

        # cur' = x @ W.T + (b + beta - 1): bias folded into contraction row
        # 1000 on the host (hi/lo split like W). k-outer; per k-tile the
        # stationary xh serves both Wh and Wl before switching to xl.
        cur = curp.tile([B, NL], F32)
        ps_tiles = [
            psum.tile([B, n1 - n0], F32, tag=f"ps{i}", name=f"ps{i}")
            for i, (n0, n1) in enumerate(MM_CHUNKS)
        ]
        n_mm = 3 * KT
        mi = 0
        for k in range(KT):
            for xt, wt in (
                (xhtile[:, k, :], wh_tiles[k]),
                (xhtile[:, k, :], wl_tiles[k]),
                (xltile[:, k, :], wh_tiles[k]),
            ):
                for i, (n0, n1) in enumerate(MM_CHUNKS):
                    nc.tensor.matmul(
                        ps_tiles[i],
                        xt,
                        wt[:, n0:n1],
                        start=(mi == 0),
                        stop=(mi == n_mm - 1),
                    )
                mi += 1
        for i, (n0, n1) in enumerate(MM_CHUNKS):
            nc.scalar.copy(out=cur[:, n0:n1], in_=ps_tiles[i])

        # v_0 = mem_0 - 1 = -1; runs on DVE during the W load.
        v0 = vpool.tile([B, NL], F32, tag="v")
        nc.vector.memset(v0, -1.0)

        # LIF scan: one fused DVE op per step. The ACT engine (0.95ns/col,
        # 59us for all 50 steps) downcasts v to fp16; one HWDGE DMA per
        # step ships it on the SP ring. Keeping the DMA fabric free of f32
        # reads stops the output stream from lagging the scan.
        v = v0
        for t in range(T):
            vn = vpool.tile([B, NL], F32, tag="v")
            nc.vector._custom_dve(
                LIF_OP, out=vn, in0=v, in1=cur, s0=BETA, s1=0.0
            )
            vh = hpool.tile([B, NL], F16, tag="vh")
            nc.scalar.copy(out=vh, in_=vn)
            nc.sync.dma_start(out=v_rec[t], in_=vh)
            v = vn

    # Raw Bass skips the extended-inst codegen pass; without it the NEFF
    # compiler sees empty .instr bytes for InstCustomDveAnt ("ISA wrong
    # length").
    from concourse.library_overlay import lower_extended_insts

    lower_extended_insts(nc)
    return _patch_serialization(nc)


_NC_CACHE = None


def _get_program() -> bass.Bass:
    global _NC_CACHE
    if _NC_CACHE is None:
        _NC_CACHE = _build_program()
    return _NC_CACHE


def _prep_inputs(x: np.ndarray, W: np.ndarray, b: np.ndarray):
    x = np.asarray(x, dtype=np.float32)
    W = np.asarray(W, dtype=np.float32)
    b = np.asarray(b, dtype=np.float32)
    bp = b + np.float32(BETA - 1.0)  # folds the v-space shift into the bias

    xT = np.zeros((K_PAD, B), dtype=np.float32)
    xT[:AXON] = x.T
    xT[AXON] = 1.0  # bias row (exact in fp16, so xl's bias row is 0)
    xh = xT.astype(np.float16)
    xl = (xT - xh.astype(np.float32)).astype(np.float16)

    wT = np.zeros((K_PAD, N_TOTAL), dtype=np.float32)
    wT[:AXON] = W.T
    wT[AXON] = bp
    wh = wT.astype(np.float16)
    wl = (wT - wh.astype(np.float32)).astype(np.float16)

    in_maps = []
    for c in range(N_CORES):
        lo, hi = c * NL, (c + 1) * NL
        in_maps.append(
            {
                "xh": xh,
                "xl": xl,
                "wh": np.ascontiguousarray(wh[:, lo:hi]),
                "wl": np.ascontiguousarray(wl[:, lo:hi]),
            }
        )
    return in_maps


def run(x, W, b, trace: bool = False):
    """Run the kernel; returns ((spk_rec, mem_rec), BassKernelResults)."""
    from concourse.bass_utils import run_bass_kernel_spmd

    nc = _get_program()
    in_maps = _prep_inputs(x, W, b)
    res = run_bass_kernel_spmd(nc, in_maps, list(range(N_CORES)), trace=trace)
    v = np.concatenate(
        [res.results[c]["v_rec"] for c in range(N_CORES)], axis=2
    ).astype(np.float32)
    spk = (v > 0).astype(np.float32)
    mem = v + np.float32(1.0)
    return (spk, mem), res


def kernel(x: np.ndarray, W: np.ndarray, b: np.ndarray):
    (spk, mem), _ = run(x, W, b)
    return spk, mem


# revision 20
# speedup vs baseline: 1.1409x; 1.1409x over previous
"""CSNN LIF-scan kernel for Trainium2, 8 NeuronCores.

reference computes:
    cur = x @ W.T + b                      # [128, 10000]
    scan t=0..49:  reset = (mem > 1); mem = 0.95*mem + cur - reset
                   spk = (mem > 1)
    returns spk_rec, mem_rec               # each [50, 128, 10000] f32

Key identities exploited here:
  * spk_rec[t] == (mem_rec[t] > 1) exactly, so only ONE tensor needs to
    leave the device; the host derives spikes from it losslessly.
  * In threshold-shifted state v = mem - 1 the whole step is
        v' = (v*beta + cur') - (v > 0),   cur' = cur + (beta - 1)
    which fits a single fused custom-DVE op (one 1x pass/step) instead of
    two scalar_tensor_tensor passes + a compare.
  * (beta-1) is folded into the matmul bias row on the host, so cur' comes
    straight out of PSUM.
  * v ships as fp16 (cast inline by the SWDGE DMA engines): v is centered
    on the spike threshold, so (v_fp16 > 0) still reproduces the spike
    train bit-exactly away from a ~2^-25 dead band.

Sharding: model-parallel over the neuron axis (10000 = 8 x 1250), x
replicated, W/b sliced per core; batch=128 rides the SBUF partitions.
"""

import sys

for _p in ("/opt/trn_rl_repo", "/root/.axon_site/_ro/trn_rl_repo"):
    if _p not in sys.path:
        sys.path.append(_p)

import numpy as np

import concourse.bass as bass
import concourse.tile as tile
import concourse.dve_ops as dve_ops
from concourse import mybir
from concourse.dve_spec import C0, C1, Spec, Src0, Src1

F32 = mybir.dt.float32
F16 = mybir.dt.float16

N_CORES = 8
B = 128          # batch (SBUF partitions)
AXON = 1000      # contraction dim
K_PAD = 1024     # padded contraction (8 x 128); row 1000 carries the bias
N_TOTAL = 10000
NL = N_TOTAL // N_CORES  # 1250 neurons per core
T = 50
BETA = 0.95
THRESH = 1.0

# matmul free-dim chunks (PSUM bank holds 512 f32)
MM_CHUNKS = [(0, 512), (512, 1024), (1024, 1250)]


def _lif_ref(in0, in1, s0, s1, imm2):
    in0 = np.asarray(in0, np.float32)
    in1 = np.asarray(in1, np.float32)
    return (
        (in0 * np.float32(s0) + in1) - (in0 > np.float32(s1)).astype(np.float32)
    ).astype(np.float32)


def _register_lif_op() -> "dve_ops.DveOp":
    """out = (in0*s0 + in1) - (in0 > s1): one fused LIF step per DVE pass."""
    name = "LIF_STEP_ANT"
    for op in dve_ops.OPS:
        if op.name == name:
            return op
    op = dve_ops.DveOp(
        name,
        Spec(body=(Src0 * C0 + Src1) - (Src0 > C1), reference=_lif_ref),
        subdim=False,
        uops_sha={"v3": "4d971942aba05d49", "v4": "da6677450a1cb1b9"},
    )
    dve_ops.OPS.append(op)
    dve_ops.CUSTOM_DVE_SPECS[name] = op.spec
    dve_ops._SUB_OPCODE_FOR_NAME[name] = (
        dve_ops._CUSTOM_DVE_ROW_BASE + len(dve_ops.OPS) - 1
    )
    assert dve_ops._SUB_OPCODE_FOR_NAME[name] < 0x20
    return op


LIF_OP = _register_lif_op()


def _split_excess_waits(bir: dict) -> int:
    """walrus in this env lowers at most ONE sync-wait per instruction, but
    Tile emits several. Move extras onto injected EventSemaphore carriers
    placed just before the instruction on the same engine."""
    n_split = [0]

    def fix_block(block):
        for inner in block.get("blocks", []):
            fix_block(inner)
        insts = block.get("instructions")
        if not insts:
            return
        new_insts = []
        for inst in insts:
            si = inst.get("sync_info")
            waits = (si or {}).get("on_wait", [])
            if len(waits) > 1:
                for w in waits[:-1]:
                    n_split[0] += 1
                    new_insts.append(
                        {
                            "debug": inst.get("debug", 0),
                            "engine": inst["engine"],
                            "ins": [],
                            "name": f"I-wsplit-{n_split[0]}",
                            "opcode": "EventSemaphore",
                            "outs": [],
                            "sync_info": {"on_update": [], "on_wait": [w]},
                        }
                    )
                si["on_wait"] = [waits[-1]]
            new_insts.append(inst)
        block["instructions"] = new_insts

    for fn in bir.get("functions", []):
        fix_block(fn)
    return n_split[0]


def _patch_serialization(nc: bass.Bass) -> bass.Bass:
    import json as _json
    import types as _types

    orig = nc.to_json_bytes

    def to_json_bytes(self):
        bir = _json.loads(orig())
        _split_excess_waits(bir)
        return _json.dumps(bir).encode()

    nc.to_json_bytes = _types.MethodType(to_json_bytes, nc)
    return nc


def _build_program() -> bass.Bass:
    from contextlib import ExitStack

    nc = bass.Bass()
    # fp16 split-matmul inputs: cur = xh@Wh + xh@Wl + xl@Wh (the dropped
    # xl@Wl term is ~2^-22 relative). PE runs fp16 at 1 cycle/row vs 4 for
    # f32, cutting the pre-scan phase by ~40%.
    xh = nc.dram_tensor("xh", [K_PAD, B], F16, kind="ExternalInput")
    xl = nc.dram_tensor("xl", [K_PAD, B], F16, kind="ExternalInput")
    wh = nc.dram_tensor("wh", [K_PAD, NL], F16, kind="ExternalInput")
    wl = nc.dram_tensor("wl", [K_PAD, NL], F16, kind="ExternalInput")
    v_rec = nc.dram_tensor("v_rec", [T, B, NL], F16, kind="ExternalOutput")

    KT = K_PAD // 128  # 8 contraction tiles

    with tile.TileContext(nc) as tc, ExitStack() as ctx:
        xpool = ctx.enter_context(tc.tile_pool(name="xp", bufs=KT))
        wpool = ctx.enter_context(tc.tile_pool(name="wp", bufs=KT))
        curp = ctx.enter_context(tc.tile_pool(name="curp", bufs=1))
        psum = ctx.enter_context(tc.tile_pool(name="psum", bufs=1, space="PSUM"))
        vpool = ctx.enter_context(tc.tile_pool(name="vp", bufs=12))
        hpool = ctx.enter_context(tc.tile_pool(name="hp", bufs=8))

        # x tensors first (feed LDWEIGHTS), then W per k-tile, hi before lo,
        # all on one HWDGE ring so arrival order matches consumption order.
        xhtile = xpool.tile([128, KT, B], F16, tag="xh")
        nc.sync.dma_start(out=xhtile, in_=xh.rearrange("(k p) m -> p k m", p=128))
        xltile = xpool.tile([128, KT, B], F16, tag="xl")
        nc.sync.dma_start(out=xltile, in_=xl.rearrange("(k p) m -> p k m", p=128))

        wh_v = wh.rearrange("(k p) n -> p k n", p=128)
        wl_v = wl.rearrange("(k p) n -> p k n", p=128)
        wh_tiles, wl_tiles = [], []
        for k in range(KT):
            wgh = wpool.tile([128, NL], F16, tag="wh")
            nc.sync.dma_start(out=wgh, in_=wh_v[:, k])
            wh_tiles.append(wgh)
            wgl = wpool.tile([128, NL], F16, tag="wl")
            nc.sync.dma_start(out=wgl, in_=wl_v[:, k])
            wl_tiles.append(wgl)

        # cur' = x @ W.T + (b + beta - 1): bias folded into contraction row
        # 1000 on the host (hi/lo split like W). k-outer; per k-tile the
        # stationary xh serves both Wh and Wl before switching to xl.
        cur = curp.tile([B, NL], F32)
        ps_tiles = [
            psum.tile([B, n1 - n0], F32, tag=f"ps{i}", name=f"ps{i}")
            for i, (n0, n1) in enumerate(MM_CHUNKS)
        ]
        n_mm = 3 * KT
        mi = 0
        for k in range(KT):
            for xt, wt in (
                (xhtile[:, k, :], wh_tiles[k]),
                (xhtile[:, k, :], wl_tiles[k]),
                (xltile[:, k, :], wh_tiles[k]),
            ):
                for i, (n0, n1) in enumerate(MM_CHUNKS):
                    nc.tensor.matmul(
                        ps_tiles[i],
                        xt,
                        wt[:, n0:n1],
                        start=(mi == 0),
                        stop=(mi == n_mm - 1),
                    )
                mi += 1
        for i, (n0, n1) in enumerate(MM_CHUNKS):
            nc.scalar.copy(out=cur[:, n0:n1], in_=ps_tiles[i])

        # v_0 = mem_0 - 1 = -1; runs on DVE during the W load.
        v0 = vpool.tile([B, NL], F32, tag="v")
        nc.vector.memset(v0, -1.0)

        # LIF scan: one fused DVE op per step. The ACT engine (0.95ns/col,
        # 59us for all 50 steps) downcasts v to fp16; one HWDGE DMA per
        # step ships it on the SP ring. Keeping the DMA fabric free of f32
        # reads stops the output stream from lagging the scan.
        v = v0
        for t in range(T):
            vn = vpool.tile([B, NL], F32, tag="v")
            nc.vector._custom_dve(
                LIF_OP, out=vn, in0=v, in1=cur, s0=BETA, s1=0.0
            )
            vh = hpool.tile([B, NL], F16, tag="vh")
            nc.scalar.copy(out=vh, in_=vn)
            # alternate the two descriptor paths (SP HWDGE / SWDGE) so a
            # throttled DMA ring doesn't back up the whole output stream
            eng = nc.sync if t % 2 == 0 else nc.gpsimd
            eng.dma_start(out=v_rec[t], in_=vh)
            v = vn

    # Raw Bass skips the extended-inst codegen pass; without it the NEFF
    # compiler sees empty .instr bytes for InstCustomDveAnt ("ISA wrong
    # length").
    from concourse.library_overlay import lower_extended_insts

    lower_extended_insts(nc)
    return _patch_serialization(nc)


_NC_CACHE = None


def _get_program() -> bass.Bass:
    global _NC_CACHE
    if _NC_CACHE is None:
        _NC_CACHE = _build_program()
    return _NC_CACHE


def _prep_inputs(x: np.ndarray, W: np.ndarray, b: np.ndarray):
    x = np.asarray(x, dtype=np.float32)
    W = np.asarray(W, dtype=np.float32)
    b = np.asarray(b, dtype=np.float32)
    bp = b + np.float32(BETA - 1.0)  # folds the v-space shift into the bias

    xT = np.zeros((K_PAD, B), dtype=np.float32)
    xT[:AXON] = x.T
    xT[AXON] = 1.0  # bias row (exact in fp16, so xl's bias row is 0)
    xh = xT.astype(np.float16)
    xl = (xT - xh.astype(np.float32)).astype(np.float16)

    wT = np.zeros((K_PAD, N_TOTAL), dtype=np.float32)
    wT[:AXON] = W.T
    wT[AXON] = bp
    wh = wT.astype(np.float16)
    wl = (wT - wh.astype(np.float32)).astype(np.float16)

    in_maps = []
    for c in range(N_CORES):
        lo, hi = c * NL, (c + 1) * NL
        in_maps.append(
            {
                "xh": xh,
                "xl": xl,
                "wh": np.ascontiguousarray(wh[:, lo:hi]),
                "wl": np.ascontiguousarray(wl[:, lo:hi]),
            }
        )
    return in_maps


def run(x, W, b, trace: bool = False):
    """Run the kernel; returns ((spk_rec, mem_rec), BassKernelResults)."""
    from concourse.bass_utils import run_bass_kernel_spmd

    nc = _get_program()
    in_maps = _prep_inputs(x, W, b)
    res = run_bass_kernel_spmd(nc, in_maps, list(range(N_CORES)), trace=trace)
    v = np.concatenate(
        [res.results[c]["v_rec"] for c in range(N_CORES)], axis=2
    ).astype(np.float32)
    spk = (v > 0).astype(np.float32)
    mem = v + np.float32(1.0)
    return (spk, mem), res


def kernel(x: np.ndarray, W: np.ndarray, b: np.ndarray):
    (spk, mem), _ = run(x, W, b)
    return spk, mem
